# revision 1
# baseline (speedup 1.0000x reference)
"""
Trainium2 Bass kernel for nn_CausalMatrixGameTransformerBlock (streaming-window attention).

Math (shapes hardcoded from the problem spec):
  B=1, S=1920 new tokens, N=12 heads, D=128, CACHE=6720,
  f=2, h=24, w=40, current_start=global_end=local_end=5760.

  With those static ints the reference reduces to:
    rq = rope(q), rk = rope(k)
    K = concat(cache_k[:, 1920:5760], rk)   # [5760, 12, 128]  (window, sink not in window)
    V = concat(cache_v[:, 1920:5760], v)
    out[q,n,:] = softmax(rq K^T / sqrt(128)) V        per head, dense over 5760 keys.

Sharding: 24 units of (head, 960-query-half); each of the 8 cores gets 3
self-contained units (its own K/V window slices).  SPMD: one Bass program,
per-core input data.

Device layout trick: q/k have their D axis pre-permuted on host to
[evens, odds] so RoPE pairs become partition blocks [0:64]/[64:128] in the
transposed layout; scores are invariant to a consistent d-permutation of
q and k, and V is left unpermuted so the output is correct untransformed.

Per unit on device:
  KT [128d, 5760kk]  (DMA'd pre-transposed; new-token cols roped on DVE)
  rqT [128d, 960q]   (roped on DVE)
  vsb [128kk, 45, 128d] (natural)
  for each 480-wide q chunk, in groups of 2 kk-tiles (software-pipelined one
  group ahead so the in-order PE never stalls on the current group's exp):
     ps[2]  = KT_t^T@rqT_chunk        (PE, f32r)  -> scores^T [kk=128, q=480] x2
     ex[2]  = exp(ps * 1/sqrt(128))   (ACT, one instruction per pair)
     exs    = ex[0] + ex[1]           (DVE pre-sum for the denominator)
     po    += vsb_t^T @ ex[i]         (PE, f32r, accumulating) -> out^T [d, q]
     pd    += ones^T @ exs            (PE, f32r, accumulating) -> denom [128, q]
  out_chunk = po * reciprocal(pd)     (DVE) -> DMA to outT
Host transposes outT back and scatters into [1,1920,12,128].
"""

import math
import numpy as np

N_CORES = 8
S = 1920
NHEADS = 12
D = 128
WIN = 5760          # attention window (keys)
KTILES = WIN // 128  # 45
UQ = 960            # queries per unit
NEW0 = 3840         # first window row that is a new (un-roped) token
QCHUNK = 480

_PROG = None


def _rope_tables():
    """cos/sin angle tables [1920, 64] exactly as the reference builds them."""
    def rope_angles(max_len, dim, theta=10000.0):
        inv = 1.0 / (theta ** (np.arange(0, dim, 2, dtype=np.float64) / dim))
        return np.outer(np.arange(max_len, dtype=np.float64), inv)

    d = D
    freqs = np.concatenate([
        rope_angles(1024, d - 4 * (d // 6)),
        rope_angles(1024, 2 * (d // 6)),
        rope_angles(1024, 2 * (d // 6)),
    ], axis=1).astype(np.float32)          # [1024, 64]

    f, h, w = 2, 24, 40
    start_frame = 6                         # current_start // (h*w) = 5760 // 960
    c = d // 2
    s0, s1 = c - 2 * (c // 3), c // 3       # 22, 21
    ang = np.concatenate([
        np.broadcast_to(freqs[start_frame:start_frame + f, :s0][:, None, None, :], (f, h, w, s0)),
        np.broadcast_to(freqs[:h, s0:s0 + s1][None, :, None, :], (f, h, w, s1)),
        np.broadcast_to(freqs[:w, s0 + s1:][None, None, :, :], (f, h, w, s1)),
    ], axis=-1).reshape(S, c)
    return np.cos(ang).astype(np.float32), np.sin(ang).astype(np.float32)


def _units_for_core(c):
    return [((u // 2), (u % 2)) for u in range(3 * c, 3 * c + 3)]


def _build_program():
    from contextlib import ExitStack
    from concourse import bacc
    import concourse.tile as tile
    import concourse.mybir as mybir

    F32 = mybir.dt.float32
    F32R = mybir.dt.float32r
    EXP = mybir.ActivationFunctionType.Exp
    SCALE = 1.0 / math.sqrt(float(D))

    nc = bacc.Bacc("TRN2", target_bir_lowering=False, debug=False,
                   enable_asserts=False, num_devices=N_CORES)

    qinT = nc.dram_tensor("qinT", [3, 128, UQ], F32, kind="ExternalInput").ap()
    qswT = nc.dram_tensor("qswT", [3, 128, UQ], F32, kind="ExternalInput").ap()
    kinT = nc.dram_tensor("kinT", [3, 128, WIN], F32, kind="ExternalInput").ap()
    kswT = nc.dram_tensor("kswT", [3, 128, S], F32, kind="ExternalInput").ap()
    vin = nc.dram_tensor("vin", [3, WIN, 128], F32, kind="ExternalInput").ap()
    ccq = nc.dram_tensor("ccq", [3, 128, UQ], F32, kind="ExternalInput").ap()
    nsq = nc.dram_tensor("nsq", [3, 128, UQ], F32, kind="ExternalInput").ap()
    cck = nc.dram_tensor("cck", [128, S], F32, kind="ExternalInput").ap()
    nsk = nc.dram_tensor("nsk", [128, S], F32, kind="ExternalInput").ap()
    onesin = nc.dram_tensor("onesin", [128, 128], F32, kind="ExternalInput").ap()
    outT = nc.dram_tensor("outT", [3, 128, UQ], F32, kind="ExternalOutput").ap()

    with ExitStack() as ctx:
        tc = ctx.enter_context(tile.TileContext(nc))
        const = ctx.enter_context(tc.tile_pool(name="const", bufs=1))
        kvpool = ctx.enter_context(tc.tile_pool(name="kv", bufs=2))
        qpool = ctx.enter_context(tc.tile_pool(name="qp", bufs=2))
        ropep = ctx.enter_context(tc.tile_pool(name="rp", bufs=1))
        expp = ctx.enter_context(tc.tile_pool(name="ex", bufs=5))
        outp = ctx.enter_context(tc.tile_pool(name="op", bufs=2))
        pss = ctx.enter_context(tc.tile_pool(name="pss", bufs=2, space="PSUM"))
        pso = ctx.enter_context(tc.tile_pool(name="pso", bufs=2, space="PSUM"))
        psd = ctx.enter_context(tc.tile_pool(name="psd", bufs=2, space="PSUM"))

        ones = const.tile([128, 128], F32R)
        cck_sb = const.tile([128, S], F32R)
        nsk_sb = const.tile([128, S], F32R)
        consts_loaded = False

        for u in range(3):
            # ---------- load + rope q^T (small, on the critical path) ----------
            qT = qpool.tile([128, UQ], F32R, name="qT")
            nc.sync.dma_start(out=qT, in_=qinT[u].bitcast(F32R))
            qsw = qpool.tile([128, UQ], F32R, name="qsw")
            nc.sync.dma_start(out=qsw, in_=qswT[u].bitcast(F32R))
            cq = qpool.tile([128, UQ], F32R, name="cq")
            nc.sync.dma_start(out=cq, in_=ccq[u].bitcast(F32R))
            sq = qpool.tile([128, UQ], F32R, name="sq")
            nc.sync.dma_start(out=sq, in_=nsq[u].bitcast(F32R))

            p1q = ropep.tile([128, UQ], F32R, name="p1q")
            nc.vector.tensor_mul(p1q, qT, cq)
            p2q = ropep.tile([128, UQ], F32R, name="p2q")
            nc.vector.tensor_mul(p2q, qsw, sq)
            rqT = qpool.tile([128, UQ], F32R, name="rqT")
            nc.vector.tensor_add(rqT, p1q, p2q)

            # ---------- load + rope K^T ----------
            # rope(x) = x*[c;c] + swap(x)*[-s;s]  (d pre-permuted evens|odds)
            KT = kvpool.tile([128, WIN], F32R, name="KT")
            vsb = kvpool.tile([128, KTILES, 128], F32R, name="vsb")
            vsrc = vin[u].rearrange("(t p) d -> p t d", p=128).bitcast(F32R)

            def _ktp(piece):
                nc.sync.dma_start(out=KT[:, piece * 1920:(piece + 1) * 1920],
                                  in_=kinT[u, :, piece * 1920:(piece + 1) * 1920].bitcast(F32R))

            def _vsp(piece):
                nc.sync.dma_start(out=vsb[:, piece * 15:(piece + 1) * 15, :],
                                  in_=vsrc[:, piece * 15:(piece + 1) * 15, :])

            _ktp(0); _vsp(0)
            if not consts_loaded:
                nc.sync.dma_start(out=ones, in_=onesin.bitcast(F32R))
                nc.sync.dma_start(out=cck_sb, in_=cck.bitcast(F32R))
                nc.sync.dma_start(out=nsk_sb, in_=nsk.bitcast(F32R))
                consts_loaded = True
            _ktp(1); _vsp(1); _ktp(2)
            ksw = ropep.tile([128, S], F32R, name="ksw")
            nc.sync.dma_start(out=ksw, in_=kswT[u].bitcast(F32R))
            _vsp(2)

            knew = KT[:, NEW0:WIN]
            p1k = ropep.tile([128, S], F32R, name="p1k")
            nc.vector.tensor_mul(p1k, knew, cck_sb)
            p2k = ropep.tile([128, S], F32R, name="p2k")
            nc.vector.tensor_mul(p2k, ksw, nsk_sb)
            nc.vector.tensor_add(knew, p1k, p2k)

            # ---------- attention ----------
            for c in range(UQ // QCHUNK):
                qs = rqT[:, c * QCHUNK:(c + 1) * QCHUNK]
                po = pso.tile([128, QCHUNK], F32, name="po")
                pd = psd.tile([128, QCHUNK], F32, name="pd")
                # one-group software pipeline: emit group g+1's QK+exp before
                # group g's PV/ones so the in-order PE never waits on exp(g)
                pend = None
                for tt in range(0, KTILES, 2):
                    nt = min(2, KTILES - tt)
                    ps = pss.tile([128, 2, 512], F32, name="ps")
                    for i in range(nt):
                        t = tt + i
                        nc.tensor.matmul(out=ps[:, i, 0:QCHUNK],
                                         lhsT=KT[:, t * 128:(t + 1) * 128],
                                         rhs=qs, start=True, stop=True)
                    ex = expp.tile([128, 2, QCHUNK], F32R, name="ex")
                    nc.scalar.activation(out=ex[:, 0:nt, :], in_=ps[:, 0:nt, 0:QCHUNK],
                                         func=EXP, scale=SCALE)
                    if nt == 2:
                        # pre-sum the pair on DVE so one denominator matmul
                        # per group suffices (halves the PE ones-pass)
                        exs = expp.tile([128, QCHUNK], F32R, name="exs", bufs=3)
                        nc.vector.tensor_add(exs, ex[:, 0, :], ex[:, 1, :])
                    else:
                        exs = ex[:, 0, :]
                    if pend is not None:
                        pex, pexs, ptt, pnt = pend
                        for i in range(pnt):
                            t = ptt + i
                            nc.tensor.matmul(out=po, lhsT=vsb[:, t, :], rhs=pex[:, i, :],
                                             start=(t == 0), stop=(t == KTILES - 1))
                        nc.tensor.matmul(out=pd, lhsT=ones, rhs=pexs,
                                         start=(ptt == 0), stop=False)
                    pend = (ex, exs, tt, nt)
                pex, pexs, ptt, pnt = pend
                for i in range(pnt):
                    t = ptt + i
                    nc.tensor.matmul(out=po, lhsT=vsb[:, t, :], rhs=pex[:, i, :],
                                     start=(t == 0), stop=(t == KTILES - 1))
                nc.tensor.matmul(out=pd, lhsT=ones, rhs=pexs,
                                 start=(ptt == 0), stop=True)
                rd = outp.tile([128, QCHUNK], F32, name="rd")
                nc.vector.reciprocal(out=rd, in_=pd)
                onrm = outp.tile([128, QCHUNK], F32, name="onrm")
                nc.vector.tensor_mul(onrm, po, rd)
                nc.sync.dma_start(out=outT[u, :, c * QCHUNK:(c + 1) * QCHUNK], in_=onrm)

    nc.compile()
    return nc


def _get_program():
    global _PROG
    if _PROG is None:
        _PROG = _build_program()
    return _PROG


def _host_prep(q, k, v, cache_k, cache_v):
    """Build the 8 per-core input maps."""
    cos, sin = _rope_tables()
    perm = np.concatenate([np.arange(0, D, 2), np.arange(1, D, 2)])

    qp = np.asarray(q, np.float32)[0][:, :, perm]              # [1920, 12, 128]
    kp = np.asarray(k, np.float32)[0][:, :, perm]
    Kfull = np.concatenate([np.asarray(cache_k, np.float32)[0, 1920:5760][:, :, perm], kp], axis=0)
    Vfull = np.concatenate([np.asarray(cache_v, np.float32)[0, 1920:5760], np.asarray(v, np.float32)[0]], axis=0)

    cosT, sinT = cos.T, sin.T                                  # [64, 1920]
    cck = np.ascontiguousarray(np.concatenate([cosT, cosT], axis=0))    # [128, 1920]
    nsk = np.ascontiguousarray(np.concatenate([-sinT, sinT], axis=0))
    swap = np.concatenate([np.arange(64, 128), np.arange(0, 64)])
    _ONES = np.ones((128, 128), np.float32)

    in_maps = []
    for c in range(N_CORES):
        units = _units_for_core(c)
        qinT = np.stack([np.ascontiguousarray(qp[half * UQ:(half + 1) * UQ, n, :].T)
                         for (n, half) in units])
        qswT = np.ascontiguousarray(qinT[:, swap, :])
        kinT = np.stack([np.ascontiguousarray(Kfull[:, n, :].T) for (n, half) in units])
        kswT = np.ascontiguousarray(kinT[:, swap, NEW0:])
        vin = np.stack([np.ascontiguousarray(Vfull[:, n, :]) for (n, half) in units])
        ccq = np.stack([cck[:, half * UQ:(half + 1) * UQ] for (n, half) in units])
        nsq = np.stack([nsk[:, half * UQ:(half + 1) * UQ] for (n, half) in units])
        in_maps.append({
            "qinT": qinT, "qswT": qswT, "kinT": kinT, "kswT": kswT,
            "vin": np.ascontiguousarray(vin),
            "ccq": np.ascontiguousarray(ccq), "nsq": np.ascontiguousarray(nsq),
            "cck": cck, "nsk": nsk, "onesin": _ONES,
        })
    return in_maps


def _gather(results):
    out = np.empty((1, S, NHEADS, D), np.float32)
    for c in range(N_CORES):
        o = results[c]["outT"]                                 # [3, 128, 960]
        for i, (n, half) in enumerate(_units_for_core(c)):
            out[0, half * UQ:(half + 1) * UQ, n, :] = o[i].T
    return out


def kernel(q, k, v, cache_k, cache_v, f=2, h=24, w=40,
           current_start=5760, global_end=5760, local_end=5760, **_extra):
    from concourse.bass_utils import run_bass_kernel_spmd

    nc = _get_program()
    in_maps = _host_prep(q, k, v, cache_k, cache_v)
    res = run_bass_kernel_spmd(nc, in_maps, list(range(N_CORES)))
    return _gather(res.results)

